# revision 5
# baseline (speedup 1.0000x reference)
"""Trainium2 Bass kernel for BitNet-style causal attention block.

Shapes (hardcoded): B=2, S=2048, H=2048, NH=NKV=16, D=128.
Sharding: 8 cores = 2 batches x 4 head-groups (4 heads each).
All heavy matmuls run in bf16 on TensorE; the int8-quantized activations and
ternary weights are exactly representable in bf16, so the quantized GEMMs are
bit-faithful with fp32 PSUM accumulation.
"""

import sys
import numpy as np

sys.path.insert(0, "/opt/trn_rl_repo")

import ml_dtypes

B, S, H = 2, 2048, 2048
NH, D = 16, 128
GH = 4              # heads per core
QKV_DIM = 3 * GH * D  # 1536 per-core qkv slice
OUTC = 512          # o_proj output columns per core
EPS = 1e-5
NT = S // 128       # 16 token tiles
NHC = H // 128      # 16 hidden chunks

_CACHE = {}


def _build():
    import concourse.mybir as mybir
    import concourse.tile as tile
    from concourse import bacc
    from concourse.masks import make_identity
    from contextlib import ExitStack

    f32 = mybir.dt.float32
    bf16 = mybir.dt.bfloat16
    i8 = mybir.dt.int8
    AX = mybir.AxisListType
    ALU = mybir.AluOpType
    AF = mybir.ActivationFunctionType

    nc = bacc.Bacc(
        "TRN2",
        target_bir_lowering=False,
        debug=False,
        enable_asserts=False,
        num_devices=8,
    )

    x_d = nc.dram_tensor("x", [S, H], f32, kind="ExternalInput").ap()
    wq_d = nc.dram_tensor("wqT", [H, QKV_DIM], f32, kind="ExternalInput").ap()
    wo_d = nc.dram_tensor("woT", [H, OUTC], f32, kind="ExternalInput").ap()
    cs_d = nc.dram_tensor("cs", [S, 2 * D], f32, kind="ExternalInput").ap()
    mk_d = nc.dram_tensor("mask", [128, 2048], bf16, kind="ExternalInput").ap()
    out_d = nc.dram_tensor("out", [S, OUTC], f32, kind="ExternalOutput").ap()

    RG = [[0, 1, 2, 3], [4, 5, 6, 7]]
    ISQ = 1.0 / float(np.sqrt(D))

    with tile.TileContext(nc) as tc:
        with ExitStack() as stk:
            def pool(name, bufs, space="SBUF"):
                return stk.enter_context(
                    tc.tile_pool(name=name, bufs=bufs, space=space))

            constp = pool("const", 1)
            dram = pool("dram", 1, "DRAM")
            wqfp = pool("wqf", 2)      # [128,768] f32 streamed (two passes)
            wofp = pool("wof", 2)      # [128,512] f32 streamed (two passes)
            wtp = pool("wt", 16)       # [128,1536] bf16 resident ternary qkv
            wotp = pool("wot", 16)     # [128,512] bf16 resident ternary o
            xfp = pool("xf", 2)        # [128,1024] f32
            xip = pool("xi", 2)        # [128,1024] i8
            xbp = pool("xb", 3)        # [128,1024] bf16
            xTp = pool("xT", 24)       # [128,128] bf16 transposed x
            csp = pool("csp", 2)       # [128,256] f32
            qkTp = pool("qkT", 8)      # [128,2048] bf16 q/k transposed slabs
            vp = pool("vp", 64)        # [128,128] bf16 v tiles
            atTp = pool("atT", 6)      # [128,512] bf16 attn out chunks
            aTp = pool("aT", 24)       # [128,128] bf16 gathered attn chunks
            ppp = pool("pp", 3)        # [128,512] bf16 probs
            rawp = pool("raw", 2)      # [128,512] f32 pre-rope q/k
            rsp = pool("rs", 2)        # [128,128] f32 rope scratch
            rdbp = pool("rdb", 2)      # [128,512] f32 bcast denom
            qbp = pool("qb", 2)        # [128,512] bf16 roped q/k
            vecp = pool("vec", 8)      # small [128,1] vectors
            outp = pool("outp", 2)     # [128,512] f32 output
            wip = pool("wi", 2)        # int8 ternarize scratch
            psb = pool("ps_big", 2, "PSUM")
            pst = pool("ps_tp", 2, "PSUM")
            pssc = pool("ps_sc", 2, "PSUM")
            psat = pool("ps_at", 1, "PSUM")
            psden = pool("ps_den", 1, "PSUM")

            # ---- constants ----
            ident = constp.tile([128, 128], bf16, tag="ident")
            make_identity(nc, ident[:])
            ones_col_b = constp.tile([128, 1], bf16, tag="ocb")
            nc.vector.memset(ones_col_b[:], 1.0)
            ones_col_f = constp.tile([128, 1], f32, tag="ocf")
            nc.vector.memset(ones_col_f[:], 1.0)
            ones_row_f = constp.tile([1, 128], f32, tag="orf")
            nc.vector.memset(ones_row_f[:], 1.0)
            mask_sb = constp.tile([128, 2048], bf16, tag="mask")
            nc.sync.dma_start(mask_sb[:], mk_d[:])
            dqx_slab = constp.tile([128, NT], f32, tag="dqx")
            sb4 = constp.tile([128, 4], f32, tag="sb4")

            # ---- P1: stream weights, |w| sums, AllReduce, scales ----
            acc_wq = vecp.tile([128, 1], f32, tag="accq")
            acc_wo = vecp.tile([128, 1], f32, tag="acco")
            for hc in range(NHC):
                for hf in range(2):
                    wqc = wqfp.tile([128, 768], f32, tag="wqf")
                    nc.sync.dma_start(
                        wqc[:], wq_d[hc * 128:(hc + 1) * 128,
                                     hf * 768:(hf + 1) * 768])
                    pa = vecp.tile([128, 1], f32, tag="pa")
                    nc.scalar.activation(wqc[:], wqc[:], AF.Abs,
                                         accum_out=pa[:])
                    if hc == 0 and hf == 0:
                        nc.vector.tensor_copy(acc_wq[:], pa[:])
                    else:
                        nc.vector.tensor_add(acc_wq[:], acc_wq[:], pa[:])
                woc = wofp.tile([128, OUTC], f32, tag="wof")
                nc.sync.dma_start(woc[:], wo_d[hc * 128:(hc + 1) * 128, :])
                pb = vecp.tile([128, 1], f32, tag="pb")
                nc.scalar.activation(woc[:], woc[:], AF.Abs, accum_out=pb[:])
                if hc == 0:
                    nc.vector.tensor_copy(acc_wo[:], pb[:])
                else:
                    nc.vector.tensor_add(acc_wo[:], acc_wo[:], pb[:])
            ps_s = pst.tile([128, 128], f32, tag="pst")
            nc.tensor.matmul(ps_s[0:1, 0:1], ones_col_f[:], acc_wq[:],
                             start=True, stop=True)
            ps_s2 = pst.tile([128, 128], f32, tag="pst")
            nc.tensor.matmul(ps_s2[0:1, 0:1], ones_col_f[:], acc_wo[:],
                             start=True, stop=True)
            sums_sb = vecp.tile([1, 2], f32, tag="sums")
            nc.scalar.activation(sums_sb[0:1, 0:1], ps_s[0:1, 0:1], AF.Copy)
            nc.scalar.activation(sums_sb[0:1, 1:2], ps_s2[0:1, 0:1], AF.Copy)
            sums_in = dram.tile([1, 2], f32, tag="sin")
            sums_out = dram.tile([1, 2], f32, tag="sout")
            nc.sync.dma_start(sums_in[:], sums_sb[:])
            nc.gpsimd.collective_compute(
                "AllReduce", ALU.add, replica_groups=RG,
                ins=[sums_in.opt()], outs=[sums_out.opt()],
            )
            arsums = vecp.tile([1, 2], f32, tag="ars")
            nc.sync.dma_start(arsums[:], sums_out[:])
            means = vecp.tile([1, 2], f32, tag="means")
            nc.vector.tensor_scalar_mul(means[0:1, 0:1], arsums[0:1, 0:1],
                                        1.0 / (6144.0 * 2048.0))
            nc.vector.tensor_scalar_mul(means[0:1, 1:2], arsums[0:1, 1:2],
                                        1.0 / (2048.0 * 2048.0))
            recips = vecp.tile([1, 2], f32, tag="recips")
            nc.vector.reciprocal(recips[:], means[:])
            pack4 = vecp.tile([1, 4], f32, tag="pack4")
            nc.vector.tensor_copy(pack4[0:1, 0:2], means[:])
            nc.vector.tensor_copy(pack4[0:1, 2:4], recips[:])
            ps_b = pst.tile([128, 128], f32, tag="pst")
            nc.tensor.matmul(ps_b[:, 0:4], ones_row_f[:], pack4[:],
                             start=True, stop=True)
            nc.scalar.activation(sb4[:], ps_b[:, 0:4], AF.Copy)
            meanwq_b = sb4[:, 0:1]
            meanwo_b = sb4[:, 1:2]
            swq_b = sb4[:, 2:3]
            swo_b = sb4[:, 3:4]

            # ---- P2: re-stream weights, ternarize to bf16 {-1,0,1} ----
            wt_t = []
            for hc in range(NHC):
                wtc = wtp.tile([128, QKV_DIM], bf16, tag="wt")
                for hf in range(2):
                    cols = slice(hf * 768, (hf + 1) * 768)
                    wqc = wqfp.tile([128, 768], f32, tag="wqf")
                    nc.sync.dma_start(
                        wqc[:], wq_d[hc * 128:(hc + 1) * 128, cols])
                    nc.vector.tensor_scalar(wqc[:], wqc[:], swq_b, 1.0,
                                            op0=ALU.mult, op1=ALU.min)
                    wi = wip.tile([128, 768], i8, tag="wi")
                    nc.vector.tensor_scalar(wi[:], wqc[:], -1.0, None,
                                            op0=ALU.max)
                    nc.vector.tensor_copy(wtc[:, cols], wi[:])
                wt_t.append(wtc)
            wot_t = []
            for hc in range(NHC):
                woc = wofp.tile([128, OUTC], f32, tag="wof")
                nc.sync.dma_start(woc[:], wo_d[hc * 128:(hc + 1) * 128, :])
                nc.vector.tensor_scalar(woc[:], woc[:], swo_b, 1.0,
                                        op0=ALU.mult, op1=ALU.min)
                wi2 = wip.tile([128, OUTC], i8, tag="wi2")
                nc.vector.tensor_scalar(wi2[:], woc[:], -1.0, None,
                                        op0=ALU.max)
                wotc = wotp.tile([128, OUTC], bf16, tag="wot")
                nc.vector.tensor_copy(wotc[:], wi2[:])
                wot_t.append(wotc)

            # ---- persistent attention slabs ----
            qT = [qkTp.tile([128, S], bf16, tag="qkT", name=f"qT{h}")
                  for h in range(GH)]
            kT = [qkTp.tile([128, S], bf16, tag="qkT", name=f"kT{h}")
                  for h in range(GH)]
            v_t = {}

            # ---- P3/P4: per token tile: quant, transpose, qkv, rope ----
            for tt in range(NT):
                row = slice(tt * 128, (tt + 1) * 128)
                xh = []
                mxh = []
                for hf in range(2):
                    xt = xfp.tile([128, 1024], f32, tag="xf")
                    nc.sync.dma_start(
                        xt[:], x_d[row, hf * 1024:(hf + 1) * 1024])
                    m = vecp.tile([128, 1], f32, tag="mxh")
                    nc.vector.tensor_reduce(m[:], xt[:], AX.X, ALU.max,
                                            apply_absolute_value=True)
                    xh.append(xt)
                    mxh.append(m)
                mx = vecp.tile([128, 1], f32, tag="mx")
                nc.vector.tensor_tensor(mx[:], mxh[0][:], mxh[1][:],
                                        op=ALU.max)
                nc.vector.tensor_scalar(mx[:], mx[:], EPS, None, op0=ALU.max)
                sx = vecp.tile([128, 1], f32, tag="sx")
                nc.vector.reciprocal(sx[:], mx[:])
                nc.vector.tensor_scalar(sx[:], sx[:], 127.0, None,
                                        op0=ALU.mult)
                nc.vector.tensor_scalar(dqx_slab[:, tt:tt + 1], mx[:],
                                        1.0 / 127.0, None, op0=ALU.mult)
                xTt = []
                for hf in range(2):
                    xi = xip.tile([128, 1024], i8, tag="xi")
                    nc.vector.tensor_scalar(xi[:], xh[hf][:], sx, None,
                                            op0=ALU.mult)
                    xb = xbp.tile([128, 1024], bf16, tag="xb")
                    nc.scalar.activation(xb[:], xi[:], AF.Copy)
                    for hh in range(8):
                        ptp = pst.tile([128, 128], bf16, tag="pst")
                        nc.tensor.transpose(
                            ptp[:], xb[:, hh * 128:(hh + 1) * 128], ident[:])
                        xc = xTp.tile([128, 128], bf16, tag="xT")
                        nc.scalar.activation(xc[:], ptp[:], AF.Copy)
                        xTt.append(xc)

                cst = csp.tile([128, 2 * D], f32, tag="cs")
                nc.sync.dma_start(cst[:], cs_d[row, :])
                dqv = vecp.tile([128, 1], f32, tag="dqv")
                nc.vector.tensor_tensor(dqv[:], dqx_slab[:, tt:tt + 1],
                                        meanwq_b, op=ALU.mult)
                dqvq = vecp.tile([128, 1], f32, tag="dqvq")
                nc.vector.tensor_scalar(dqvq[:], dqv[:], ISQ, None,
                                        op0=ALU.mult)

                for dg in range(3):
                    ps = psb.tile([128, 512], f32, tag="psb")
                    for hc in range(NHC):
                        nc.tensor.matmul(
                            ps[:],
                            xTt[hc][:],
                            wt_t[hc][:, dg * 512:(dg + 1) * 512],
                            start=(hc == 0), stop=(hc == NHC - 1),
                        )
                    if dg < 2:  # q or k: dequant fp32 + rope -> bf16
                        raw = rawp.tile([128, 512], f32, tag="raw")
                        nc.scalar.activation(raw[:], ps[:], AF.Copy,
                                             scale=(dqvq if dg == 0 else dqv))
                        qb = qbp.tile([128, 512], bf16, tag="qb")
                        for h in range(GH):
                            b0 = h * 128
                            rot = rsp.tile([128, 128], f32, tag="rot")
                            nc.vector.tensor_scalar_mul(
                                rot[:, 0:64], raw[:, b0 + 64:b0 + 128], -1.0)
                            nc.vector.tensor_copy(
                                rot[:, 64:128], raw[:, b0:b0 + 64])
                            m1 = rsp.tile([128, 128], f32, tag="m1")
                            nc.vector.tensor_tensor(
                                m1[:], raw[:, b0:b0 + 128], cst[:, 0:D],
                                op=ALU.mult)
                            m2 = rsp.tile([128, 128], f32, tag="m2")
                            nc.vector.tensor_tensor(
                                m2[:], rot[:], cst[:, D:2 * D], op=ALU.mult)
                            nc.vector.tensor_tensor(
                                qb[:, b0:b0 + 128], m1[:], m2[:], op=ALU.add)
                        dst = qT if dg == 0 else kT
                        for h in range(GH):
                            ptp = pst.tile([128, 128], bf16, tag="pst")
                            nc.tensor.transpose(
                                ptp[:], qb[:, h * 128:(h + 1) * 128], ident[:])
                            nc.scalar.activation(
                                dst[h][:, row], ptp[:], AF.Copy)
                    else:  # v: dequant straight to bf16 [tok, d] tiles
                        for h in range(GH):
                            vt = vp.tile([128, 128], bf16, tag="vp")
                            nc.scalar.activation(
                                vt[:], ps[:, h * 128:(h + 1) * 128], AF.Copy,
                                scale=dqv)
                            v_t[(tt, h)] = vt

            # ---- P5: attention per head; P6 AllGather inputs streamed ----
            ag_in = dram.tile([GH * 128, S], bf16, tag="agin")
            ag_out = dram.tile([NH * 128, S], bf16, tag="agout")
            for h in range(GH):
                for qt in range(4):
                    at_ps = psat.tile([128, 512], f32, tag="psat")
                    den_ps = psden.tile([128, 512], f32, tag="psden")
                    nkc = 4 * qt + 4
                    for kc in range(nkc):
                        sc_ps = pssc.tile([128, 512], f32, tag="pssc")
                        nc.tensor.matmul(
                            sc_ps[:],
                            kT[h][:, kc * 128:(kc + 1) * 128],
                            qT[h][:, qt * 512:(qt + 1) * 512],
                            start=True, stop=True,
                        )
                        p = ppp.tile([128, 512], bf16, tag="pp")
                        nc.scalar.activation(p[:], sc_ps[:], AF.Exp)
                        dd = kc - 4 * qt
                        if dd >= 0:
                            nc.vector.tensor_tensor(
                                p[:], p[:],
                                mask_sb[:, dd * 512:(dd + 1) * 512],
                                op=ALU.mult)
                        nc.tensor.matmul(at_ps[:], v_t[(kc, h)][:], p[:],
                                         start=(kc == 0), stop=(kc == nkc - 1))
                        nc.tensor.matmul(den_ps[0:1, :], ones_col_b[:], p[:],
                                         start=(kc == 0), stop=(kc == nkc - 1))
                    rden = vecp.tile([1, 512], f32, tag="rden")
                    nc.vector.reciprocal(rden[:], den_ps[0:1, :])
                    rdb = rdbp.tile([128, 512], f32, tag="rdb")
                    nc.gpsimd.partition_broadcast(rdb[:], rden[:])
                    atile = atTp.tile([128, 512], bf16, tag="atT")
                    nc.vector.tensor_tensor(atile[:], at_ps[:], rdb[:],
                                            op=ALU.mult)
                    nc.sync.dma_start(
                        ag_in[h * 128:(h + 1) * 128,
                              qt * 512:(qt + 1) * 512], atile[:])
            nc.gpsimd.collective_compute(
                "AllGather", ALU.bypass, replica_groups=RG,
                ins=[ag_in.opt()], outs=[ag_out.opt()],
            )

            # ---- P7: o_proj ----
            for tt in range(NT):
                row = slice(tt * 128, (tt + 1) * 128)
                ps = psb.tile([128, 512], f32, tag="psb")
                for ac in range(16):
                    a = aTp.tile([128, 128], bf16, tag="aT")
                    nc.sync.dma_start(
                        a[:], ag_out[ac * 128:(ac + 1) * 128, row])
                    nc.tensor.matmul(
                        ps[:], a[:], wot_t[ac][:],
                        start=(ac == 0), stop=(ac == 15),
                    )
                ot = outp.tile([128, OUTC], f32, tag="outp")
                nc.scalar.activation(ot[:], ps[:], AF.Copy, scale=meanwo_b)
                nc.sync.dma_start(out_d[row, :], ot[:])

    nc.compile()
    return nc


def _get_nc():
    if "nc" not in _CACHE:
        _CACHE["nc"] = _build()
    return _CACHE["nc"]


def _prep_inputs(hidden_states, cos, sin, w_qkv, w_o):
    bf16 = ml_dtypes.bfloat16
    cs = np.ascontiguousarray(
        np.concatenate([cos, sin], axis=1).astype(np.float32))
    kp = np.arange(128)[:, None]
    qf = np.arange(512)[None, :]
    mask = np.concatenate(
        [(d + kp <= qf) for d in (0, 128, 256, 384)], axis=1
    ).astype(bf16)
    in_maps = []
    for c in range(8):
        b, g = c // 4, c % 4
        r0 = 512 * g
        wq_sl = np.concatenate([
            w_qkv[r0:r0 + 512],
            w_qkv[2048 + r0:2048 + r0 + 512],
            w_qkv[4096 + r0:4096 + r0 + 512],
        ], axis=0)
        in_maps.append({
            "x": np.ascontiguousarray(hidden_states[b].astype(np.float32)),
            "wqT": np.ascontiguousarray(wq_sl.T.astype(np.float32)),
            "woT": np.ascontiguousarray(w_o[r0:r0 + 512].T.astype(np.float32)),
            "cs": cs,
            "mask": mask,
        })
    return in_maps


def kernel(hidden_states, cos, sin, w_qkv, w_o, _want_trace=False):
    from concourse import bass_utils

    hidden_states = np.asarray(hidden_states, dtype=np.float32)
    cos = np.asarray(cos, dtype=np.float32)
    sin = np.asarray(sin, dtype=np.float32)
    w_qkv = np.asarray(w_qkv, dtype=np.float32)
    w_o = np.asarray(w_o, dtype=np.float32)

    nc = _get_nc()
    in_maps = _prep_inputs(hidden_states, cos, sin, w_qkv, w_o)
    res = bass_utils.run_bass_kernel_spmd(
        nc, in_maps, core_ids=list(range(8)), trace=_want_trace)
    _CACHE["last_results"] = res
    out = np.empty((B, S, NH * D), dtype=np.float32)
    for c in range(8):
        b, g = c // 4, c % 4
        out[b, :, 512 * g:512 * (g + 1)] = res.results[c]["out"]
    return out
